# revision 1
# baseline (speedup 1.0000x reference)
"""DiscreteKeyValueBottleneck Trainium2 kernel.

Reference computation (per batch b, codebook c, token t):
  idx = argmin_k ||batch[b,c,t,:] - keys[c,k,:]||^2
  mapped[b,c,t,:] = values[c, idx, :]
  pooled = mean_c mapped               -> [B, T, V]
  out = softmax(pooled, axis=T)        -> [B, T, V]

Sharding: one codebook per NeuronCore (expert-style, C == 8 == n_cores).
Each core computes its codebook's mapped values for ALL batches, a
ReduceScatter(add) over the batch axis combines codebooks and leaves each
core with 2 batches, on which it runs the softmax locally.

argmin is computed as argmax_k (x.k - |k|^2/2) via a 65-row contraction:
row 64 of the stationary operand is 1.0 and row 64 of the moving operand
is -|k|^2/2, so PSUM holds the final scores directly (fp32, error ~5e-6,
safely below the 6e-5 min top-2 gap of this input distribution).
"""

import numpy as np

B, C, T, D = 16, 8, 256, 64
K, V = 4096, 64
NCORES = 8
NT = B * T            # tokens per core (all batches, one codebook)
NTILES = NT // 128    # 32 token tiles
NCHUNK = K // 512     # 8 key chunks (one PSUM bank each)
BSH = B // NCORES     # batches per core after reduce-scatter

# offload part of the per-token max computation to the GPSIMD engine
# GPSIMD elementwise ops fail to compile on the bass2jax/axon execution
# path, so the max-tree offload stays disabled; DVE max8 is the verified path.
GPSIMD_MAX_TREE = False

_prog_cache = {}


def _build_program(single_core_sim=False):
    import concourse.bass as bass
    import concourse.tile as tile
    from concourse import bacc, mybir

    nc = bacc.Bacc('TRN2', target_bir_lowering=False, debug=False,
                   num_devices=1 if single_core_sim else NCORES)
    f32 = mybir.dt.float32

    xb = nc.dram_tensor('xb', [NT, D], f32, kind='ExternalInput').ap()
    keys = nc.dram_tensor('keys', [K, D], f32, kind='ExternalInput').ap()
    values = nc.dram_tensor('values', [K, V], f32, kind='ExternalInput').ap()
    ident_in = nc.dram_tensor('ident', [128, 128], f32, kind='ExternalInput').ap()
    out = nc.dram_tensor('out', [BSH * T, V], f32, kind='ExternalOutput').ap()

    # two half-size bounce buffers so the first ReduceScatter (batches 0-7)
    # can launch while the second half of the main loop is still running;
    # each core ends up with batches {i, i+8} (host reorders).
    partial_a = nc.dram_tensor('partial_a', [NT // 2, V], f32).ap()
    partial_b = nc.dram_tensor('partial_b', [NT // 2, V], f32).ap()
    rs_a = nc.dram_tensor('rs_a', [T, V], f32).ap()
    rs_b = nc.dram_tensor('rs_b', [T, V], f32).ap()

    with tile.TileContext(nc) as tc:
        with (
            tc.tile_pool(name='const', bufs=1) as constp,
            tc.tile_pool(name='loads', bufs=3) as loads,
            tc.tile_pool(name='scores', bufs=3) as scoresp,
            tc.tile_pool(name='small', bufs=2) as smallp,
            tc.tile_pool(name='tail', bufs=1) as tailp,
            tc.tile_pool(name='ptr', bufs=2, space='PSUM') as ptr,
            tc.tile_pool(name='pmain', bufs=4, space='PSUM') as pmain,
        ):
            ident = constp.tile([128, 128], f32)
            nc.sync.dma_start(ident[:], ident_in[:])

            # ---- keys prep (per chunk, so main MMs can start early) ----
            # keysc[j][0:64] = keys^T chunk j, row 64 = -|k|^2/2
            ones64 = constp.tile([64, 1], f32)
            nc.vector.memset(ones64[:], 1.0)
            keysc = []
            for j in range(NCHUNK):
                kc = constp.tile([65, 512], f32, tag=f'keysc{j}')
                keysc.append(kc)
                kld = loads.tile([128, 4 * D], f32, tag='kld')
                for q in range(4):
                    nc.sync.dma_start(
                        kld[:, q * D:(q + 1) * D],
                        keys[j * 512 + q * 128: j * 512 + (q + 1) * 128, :])
                pt = ptr.tile([64, 512], f32, tag='tr')
                for q in range(4):
                    nc.tensor.transpose(
                        pt[:, q * 128:(q + 1) * 128], kld[:, q * D:(q + 1) * D],
                        ident[:])
                nc.scalar.copy(kc[0:64, :], pt[:])
                # square and k2-evict run on the DVE (idle during the head);
                # keeping them off ACT shortens the serial keys-prep chain
                # that gates the first main matmul.
                sqj = loads.tile([64, 512], f32, tag='sqj')
                nc.vector.tensor_mul(sqj[:], kc[0:64, :], kc[0:64, :])
                pk2 = ptr.tile([1, 512], f32, tag='k2')
                nc.tensor.matmul(pk2[:], ones64[:], sqj[:],
                                 start=True, stop=True)
                nc.vector.tensor_scalar_mul(kc[64:65, :], pk2[:], -0.5)

            # ---- X prep (per token tile): xt[0:64] = X_k^T, row 64 = 1 ----
            xts = []
            for k in range(NTILES):
                xt = constp.tile([65, 128], f32, tag=f'xt{k}')
                xts.append(xt)
                nc.vector.memset(xt[64:65, :], 1.0)
                xld = loads.tile([128, D], f32, tag='xld')
                nc.sync.dma_start(xld[:], xb[k * 128:(k + 1) * 128, :])
                ptx = ptr.tile([64, 128], f32, tag='tr')
                nc.tensor.transpose(ptx[:], xld[:], ident[:])
                nc.scalar.copy(xt[0:64, :], ptx[:])

            # ---- main: scores + argmax + values-gather per token tile ----
            for k in range(NTILES):
                scores = scoresp.tile([128, K], f32, tag='scores')
                for j in range(NCHUNK):
                    pm = pmain.tile([128, 512], f32, tag='mm')
                    nc.tensor.matmul(
                        pm[:], xts[k][:], keysc[j][:],
                        start=True, stop=True)
                    nc.scalar.copy(scores[:, j * 512:(j + 1) * 512], pm[:])
                if GPSIMD_MAX_TREE:
                    # elementwise max-tree over the 8 chunks, mostly on the
                    # otherwise-idle GPSIMD engine; DVE only reduces the last
                    # 512-wide slab and runs max_index.
                    t4 = smallp.tile([128, 4 * 512], f32, tag='t4')
                    for h in range(4):
                        nc.gpsimd.tensor_max(
                            t4[:, h * 512:(h + 1) * 512],
                            scores[:, (2 * h) * 512:(2 * h + 1) * 512],
                            scores[:, (2 * h + 1) * 512:(2 * h + 2) * 512])
                    t2 = smallp.tile([128, 2 * 512], f32, tag='t2')
                    nc.gpsimd.tensor_max(t2[:, 0:512], t4[:, 0:512],
                                         t4[:, 512:1024])
                    nc.vector.tensor_max(t2[:, 512:1024], t4[:, 1024:1536],
                                         t4[:, 1536:2048])
                    t1 = smallp.tile([128, 512], f32, tag='t1')
                    nc.vector.tensor_max(t1[:], t2[:, 0:512], t2[:, 512:1024])
                    g = smallp.tile([128, 1], f32, tag='g')
                    nc.vector.tensor_reduce(g[:], t1[:],
                                            op=mybir.AluOpType.max,
                                            axis=mybir.AxisListType.X)
                    idx8 = smallp.tile([128, 8], mybir.dt.uint32, tag='idx8')
                    nc.vector.max_index(
                        idx8[:], g[:, 0:1].to_broadcast([128, 8]), scores[:])
                else:
                    mx8 = smallp.tile([128, 8], f32, tag='mx8')
                    nc.vector.max(mx8[:], scores[:])
                    idx8 = smallp.tile([128, 8], mybir.dt.uint32, tag='idx8')
                    nc.vector.max_index(idx8[:], mx8[:], scores[:])
                mapped = smallp.tile([128, V], f32, tag='mapped')
                nc.gpsimd.indirect_dma_start(
                    out=mapped[:], out_offset=None, in_=values[:],
                    in_offset=bass.IndirectOffsetOnAxis(ap=idx8[:, 0:1], axis=0))
                if k < NTILES // 2:
                    nc.sync.dma_start(partial_a[k * 128:(k + 1) * 128, :],
                                      mapped[:])
                else:
                    kk = k - NTILES // 2
                    nc.sync.dma_start(partial_b[kk * 128:(kk + 1) * 128, :],
                                      mapped[:])
                if not single_core_sim and k == NTILES // 2 - 1:
                    nc.gpsimd.collective_compute(
                        'ReduceScatter', mybir.AluOpType.add,
                        replica_groups=[list(range(NCORES))],
                        ins=[partial_a[:]], outs=[rs_a[:]])

            # ---- combine codebooks: second-half ReduceScatter ----
            if single_core_sim:
                # TimelineSim can't simulate collectives; stand in same-size
                # local copies so the tail still gets modeled.
                cp = tailp.tile([128, BSH * T // 128 * V], f32, tag='rscopy')
                for q in range(2):
                    nc.sync.dma_start(cp[:, q * V:(q + 1) * V],
                                      partial_a[q * 128:(q + 1) * 128, :])
                    nc.sync.dma_start(cp[:, (q + 2) * V:(q + 3) * V],
                                      partial_b[q * 128:(q + 1) * 128, :])
                for q in range(2):
                    nc.sync.dma_start(rs_a[q * 128:(q + 1) * 128, :],
                                      cp[:, q * V:(q + 1) * V])
                    nc.sync.dma_start(rs_b[q * 128:(q + 1) * 128, :],
                                      cp[:, (q + 2) * V:(q + 3) * V])
            else:
                nc.gpsimd.collective_compute(
                    'ReduceScatter', mybir.AluOpType.add,
                    replica_groups=[list(range(NCORES))],
                    ins=[partial_b[:]], outs=[rs_b[:]])

            # ---- softmax over T per (batch, v) on the local 2-batch shard --
            pts = ptr.tile([64, BSH * T], f32, tag='tr')
            for q in range(BSH * T // 128):
                sld = loads.tile([128, V], f32, tag='sld')
                rs_src = rs_a if q < 2 else rs_b
                nc.sync.dma_start(sld[:],
                                  rs_src[(q % 2) * 128:(q % 2 + 1) * 128, :])
                nc.tensor.transpose(pts[:, q * 128:(q + 1) * 128], sld[:],
                                    ident[:])
            sm = tailp.tile([64, BSH * T], f32)
            den = smallp.tile([64, BSH], f32, tag='den')
            for b in range(BSH):
                nc.scalar.activation(
                    sm[:, b * T:(b + 1) * T], pts[:, b * T:(b + 1) * T],
                    mybir.ActivationFunctionType.Exp,
                    scale=1.0 / C, accum_out=den[:, b:b + 1])
            rden = smallp.tile([64, BSH], f32, tag='rden')
            nc.vector.reciprocal(rden[:], den[:])
            for b in range(BSH):
                nc.vector.tensor_scalar(
                    out=sm[:, b * T:(b + 1) * T], in0=sm[:, b * T:(b + 1) * T],
                    scalar1=rden[:, b:b + 1], scalar2=None,
                    op0=mybir.AluOpType.mult)
            pso = ptr.tile([128, BSH * T // 128 * V], f32, tag='tr')
            so = tailp.tile([128, BSH * T // 128 * V], f32)
            for q in range(BSH * T // 128):
                nc.tensor.transpose(pso[:, q * V:(q + 1) * V],
                                    sm[:, q * 128:(q + 1) * 128],
                                    ident[0:64, 0:64])
            nc.scalar.copy(so[:], pso[:])
            for q in range(BSH * T // 128):
                nc.sync.dma_start(out[q * 128:(q + 1) * 128, :],
                                  so[:, q * V:(q + 1) * V])

    nc.compile()
    return nc


def _get_program():
    if 'nc' not in _prog_cache:
        _prog_cache['nc'] = _build_program()
    return _prog_cache['nc']


def kernel(batch, keys, values):
    from concourse import bass_utils

    nc = _get_program()
    ident = np.eye(128, dtype=np.float32)
    in_maps = []
    for c in range(NCORES):
        in_maps.append({
            'xb': np.ascontiguousarray(
                batch[:, c].reshape(NT, D).astype(np.float32)),
            'keys': np.ascontiguousarray(keys[c].astype(np.float32)),
            'values': np.ascontiguousarray(values[c].astype(np.float32)),
            'ident': ident,
        })
    res = bass_utils.run_bass_kernel_spmd(nc, in_maps,
                                          core_ids=list(range(NCORES)))
    # core i holds batches {i, i + 8} (split reduce-scatter halves)
    out = np.empty((B, T, V), dtype=np.float32)
    for i in range(NCORES):
        shard = res.results[i]['out'].reshape(BSH, T, V)
        out[i] = shard[0]
        out[i + NCORES] = shard[1]
    return out



# revision 5
# speedup vs baseline: 1.6690x; 1.6690x over previous
"""DiscreteKeyValueBottleneck Trainium2 kernel.

Reference computation (per batch b, codebook c, token t):
  idx = argmin_k ||batch[b,c,t,:] - keys[c,k,:]||^2
  mapped[b,c,t,:] = values[c, idx, :]
  pooled = mean_c mapped               -> [B, T, V]
  out = softmax(pooled, axis=T)        -> [B, T, V]

Sharding: one codebook per NeuronCore (expert-style, C == 8 == n_cores).
Each core computes its codebook's mapped values for ALL batches, a
ReduceScatter(add) over the batch axis combines codebooks and leaves each
core with 2 batches, on which it runs the softmax locally.

argmin is computed as argmax_k (x.k - |k|^2/2).  The scores matmul runs
in fp32r (fp32 with 12 mantissa bits dropped, 4x faster on the PE) using
an exact 2-term split: x = x_r + x_e and k = k_r + k_e with _r/_e both
fp32r-representable, so
  x.k = [x_r;x_e].[k_r;k_r]  (MM1, 128-row contraction)
      + [x_r;1;1].[k_e;k2_r;k2_e]  (MM2, 66-row, accumulated in PSUM)
dropping only x_e.k_e ~ 2^-24 (total score error ~5e-6, far below the
~6e-5 min top-2 gap of this input distribution).  The r/e splitting and
operand transposes are host-side layout; |k|^2 is computed on device.

argmax runs as ONE custom DVE instruction per token tile (ARGMAX2_ANT):
in0/in1 = even/odd elements of the score row, a scan(MAX) prefix max,
winner-index encoding 2*Idx + (odd wins), accum(MAX) -> the argmax index
in a single 2048-column pass (vs 8192 columns for max8 + max_index).
"""

import numpy as np

B, C, T, D = 16, 8, 256, 64
K, V = 4096, 64
NCORES = 8
NT = B * T            # tokens per core (all batches, one codebook)
NTILES = NT // 128    # 32 token tiles
NCHUNK = K // 512     # 8 key chunks
BSH = B // NCORES     # batches per core after reduce-scatter

_prog_cache = {}


# ---------------------------------------------------------------------------
# custom DVE op: single-pass argmax over even/odd interleaved halves
# ---------------------------------------------------------------------------

def _ref_argmax2(in0, in1, c0, c1, c2):
    P = in0.shape[0]
    a = np.asarray(in0, np.float32).reshape(P, -1)
    b = np.asarray(in1, np.float32).reshape(P, -1)
    w = np.maximum(a, b)
    m = np.maximum.accumulate(w, axis=1)
    tag = (b > a).astype(np.float32)
    step = float(np.asarray(c0).flat[0]) if np.ndim(c0) else float(c0)
    idx2 = step * np.arange(w.shape[1], dtype=np.float32)[None, :]
    wi = idx2 + tag
    body = np.where(w >= m, wi, np.float32(np.finfo(np.float32).min))
    return body.astype(np.float32), body.max(axis=1, keepdims=True)


def _get_argmax_op():
    if 'op' in _prog_cache:
        return _prog_cache['op']
    from concourse.dve_spec import (
        Spec, Src0, Src1, C0, Zero, MaxNeg, AluOp, Bin, scan, select, maxx)
    from concourse.dve_ops import (
        DveOp, OPS, CUSTOM_DVE_SPECS, _SUB_OPCODE_FOR_NAME)
    from concourse.dve_uop import DveOpSpec
    from concourse.dve_spec import lower, _has_src1

    w = maxx(Src0, Src1)
    m = scan(AluOp.MAX, w)
    tag = Bin(AluOp.IS_LT, Src0, Src1)
    idx2 = scan(AluOp.ADD, C0, init=Bin(AluOp.SUBTRACT, Zero, C0))
    body = select(Bin(AluOp.IS_GE, w, m), idx2 + tag, MaxNeg)
    op = DveOp('ARGMAX2_ANT',
               Spec(body=body, accum=maxx, accum_init=MaxNeg,
                    reference=_ref_argmax2),
               subdim=False, uops_sha={})
    if op.name not in _SUB_OPCODE_FOR_NAME:
        OPS.append(op)
        CUSTOM_DVE_SPECS[op.name] = op.spec
        _SUB_OPCODE_FOR_NAME[op.name] = max(_SUB_OPCODE_FOR_NAME.values()) + 1
        for ver in ('v3', 'v4'):
            s = DveOpSpec(name=op.name, opcode=_SUB_OPCODE_FOR_NAME[op.name],
                          uops=lower(op.spec, ver=ver),
                          rd1_en=_has_src1(op.spec))
            op.uops_sha[ver] = s.sha(ver)
    _prog_cache['op'] = op
    return op


# ---------------------------------------------------------------------------
# bass program
# ---------------------------------------------------------------------------

def _build_program(single_core_sim=False):
    import concourse.bass as bass
    import concourse.tile as tile
    from concourse import bacc, mybir

    ARGMAX2 = _get_argmax_op()

    nc = bacc.Bacc('TRN2', target_bir_lowering=False, debug=False,
                   num_devices=1 if single_core_sim else NCORES)
    f32 = mybir.dt.float32
    f32r = mybir.dt.float32r
    u32 = mybir.dt.uint32

    # host-prepared, transposed + fp32r-split operands (see kernel()):
    #   xsplit  [128, NT]: rows 0-63 x_r^T, rows 64-127 x_e^T   (fp32r bits)
    #   xt2src  [66, NT]:  rows 0-63 x_r^T, rows 64-65 = 1.0
    #   krdup   [128, K]:  rows 0-63 k_r^T, rows 64-127 k_r^T
    #   kesrc   [64, K]:   k_e^T
    #   ksq     [64, K]:   (k^T)^2  (fp32; for the on-device |k|^2 matmul)
    xsplit_d = nc.dram_tensor('xsplit', [128, NT], f32r, kind='ExternalInput').ap()
    xt2_d = nc.dram_tensor('xt2src', [66, NT], f32r, kind='ExternalInput').ap()
    krdup_d = nc.dram_tensor('krdup', [128, K], f32r, kind='ExternalInput').ap()
    kesrc_d = nc.dram_tensor('kesrc', [64, K], f32r, kind='ExternalInput').ap()
    ksq_d = nc.dram_tensor('ksq', [64, K], f32, kind='ExternalInput').ap()
    values = nc.dram_tensor('values', [K, V], f32, kind='ExternalInput').ap()
    ident_in = nc.dram_tensor('ident', [128, 128], f32, kind='ExternalInput').ap()
    out = nc.dram_tensor('out', [BSH * T, V], f32, kind='ExternalOutput').ap()

    # two half-size bounce buffers so the first ReduceScatter (batches 0-7)
    # can launch while the second half of the main loop is still running;
    # each core ends up with batches {i, i+8} (host reorders).
    partial_a = nc.dram_tensor('partial_a', [NT // 2, V], f32).ap()
    partial_b = nc.dram_tensor('partial_b', [NT // 2, V], f32).ap()
    rs_a = nc.dram_tensor('rs_a', [T, V], f32).ap()
    rs_b = nc.dram_tensor('rs_b', [T, V], f32).ap()

    with tile.TileContext(nc) as tc:
        with (
            tc.tile_pool(name='const', bufs=1) as constp,
            tc.tile_pool(name='loads', bufs=3) as loads,
            tc.tile_pool(name='scores', bufs=3) as scoresp,
            tc.tile_pool(name='small', bufs=3) as smallp,
            tc.tile_pool(name='junk', bufs=1) as junkp,
            tc.tile_pool(name='prep', bufs=1) as prepp,
            tc.tile_pool(name='tail', bufs=1) as tailp,
        ):
            # ---- bulk operand loads (one DMA each) ----
            xsp = constp.tile([128, NT], f32r)
            nc.sync.dma_start(xsp[:], xsplit_d[:])
            xt2 = constp.tile([66, NT], f32r)
            nc.sync.dma_start(xt2[:], xt2_d[:])
            krd = constp.tile([128, K], f32r)
            nc.sync.dma_start(krd[:], krdup_d[:])
            rhs2 = constp.tile([66, K], f32r)
            nc.sync.dma_start(rhs2[0:64, :], kesrc_d[:])
            ksq = constp.tile([64, K], f32)
            nc.sync.dma_start(ksq[:], ksq_d[:])
            ident = constp.tile([128, 128], f32)
            nc.sync.dma_start(ident[:], ident_in[:])
            ones64 = constp.tile([64, 1], f32)
            nc.vector.memset(ones64[:], 1.0)
            idxall = constp.tile([128, NTILES], f32)

            with tc.tile_pool(name='pk2', bufs=2, space='PSUM') as pk2p:
                # ---- |k|^2 rows of rhs2 (device compute) ----
                # per-chunk [1,512] matmul (PSUM matmul out must start at
                # partition 0), ACT-copy into one [1, K] row, then reshape to
                # [8, 512] via a cross-partition SBUF DMA so the split ops run
                # 512 columns wide instead of 4096.
                k2row = prepp.tile([1, K], f32, tag='k2row')
                for j in range(NCHUNK):
                    pk2 = pk2p.tile([1, 512], f32, tag='k2')
                    nc.tensor.matmul(pk2[:], ones64[:],
                                     ksq[:, j * 512:(j + 1) * 512],
                                     start=True, stop=True)
                    nc.scalar.copy(k2row[:, j * 512:(j + 1) * 512], pk2[:])
                k2st = prepp.tile([NCHUNK, 512], f32, tag='k2st')
                nc.sync.dma_start(k2st[:], k2row[:])
                k2all = prepp.tile([NCHUNK, 512], f32, tag='k2all')
                nc.vector.tensor_scalar_mul(k2all[:], k2st[:], -0.5)
                k2r = prepp.tile([NCHUNK, 512], f32r, tag='k2r')
                nc.vector.tensor_copy(out=k2r[:], in_=k2all[:])
                k2e = prepp.tile([NCHUNK, 512], f32, tag='k2e')
                nc.vector.tensor_tensor(out=k2e[:], in0=k2all[:],
                                        in1=k2r[:].bitcast(f32),
                                        op=mybir.AluOpType.subtract)
                k2er = prepp.tile([NCHUNK, 512], f32r, tag='k2er')
                nc.vector.tensor_copy(out=k2er[:], in_=k2e[:])
                # scatter [8,512] partition-rows into rhs2 rows 64 / 65
                # (dma_start only requires equal element counts; the AP
                # balancer reconciles [8,512] -> [1,4096])
                nc.sync.dma_start(rhs2[64:65, :], k2r[:])
                nc.sync.dma_start(rhs2[65:66, :], k2er[:])

            # ---- main loop ----
            with tc.tile_pool(name='pmm', bufs=1, space='PSUM') as pmm:
                for t in range(NTILES):
                    scores = scoresp.tile([128, K], f32, tag='scores')
                    lhsT1 = xsp[:, t * 128:(t + 1) * 128]
                    lhsT2 = xt2[:, t * 128:(t + 1) * 128]
                    for h in range(2):  # half = 4 chunks = one PSUM group
                        pm = pmm.tile([128, 2048], f32, tag=f'mm{h}')
                        for jj in range(4):
                            j = 4 * h + jj
                            nc.tensor.matmul(
                                pm[:, jj * 512:(jj + 1) * 512], lhsT1,
                                krd[:, j * 512:(j + 1) * 512],
                                start=True, stop=False)
                            nc.tensor.matmul(
                                pm[:, jj * 512:(jj + 1) * 512], lhsT2,
                                rhs2[:, j * 512:(j + 1) * 512],
                                start=False, stop=True)
                        nc.scalar.copy(scores[:, h * 2048:(h + 1) * 2048],
                                       pm[:])

                    junk = junkp.tile([128, K // 2], f32)
                    nc.vector._custom_dve(
                        ARGMAX2, out=junk[:],
                        in0=scores[:, 0:K:2], in1=scores[:, 1:K:2],
                        s0=2.0, accum_out=idxall[:, t:t + 1])
                    idxu = smallp.tile([128, 1], u32, tag='idxu')
                    nc.vector.tensor_copy(out=idxu[:], in_=idxall[:, t:t + 1])
                    mapped = smallp.tile([128, V], f32, tag='mapped')
                    nc.gpsimd.indirect_dma_start(
                        out=mapped[:], out_offset=None, in_=values[:],
                        in_offset=bass.IndirectOffsetOnAxis(ap=idxu[:], axis=0))
                    if t < NTILES // 2:
                        nc.sync.dma_start(
                            partial_a[t * 128:(t + 1) * 128, :], mapped[:])
                    else:
                        tt = t - NTILES // 2
                        nc.sync.dma_start(
                            partial_b[tt * 128:(tt + 1) * 128, :], mapped[:])
                    if not single_core_sim and t == NTILES // 2 - 1:
                        nc.gpsimd.collective_compute(
                            'ReduceScatter', mybir.AluOpType.add,
                            replica_groups=[list(range(NCORES))],
                            ins=[partial_a[:]], outs=[rs_a[:]])

            # ---- combine codebooks: second-half ReduceScatter ----
            if single_core_sim:
                # TimelineSim can't simulate collectives; stand in same-size
                # local copies so the tail still gets modeled.
                cp = tailp.tile([128, BSH * T // 128 * V], f32, tag='rscopy')
                for q in range(2):
                    nc.sync.dma_start(cp[:, q * V:(q + 1) * V],
                                      partial_a[q * 128:(q + 1) * 128, :])
                    nc.sync.dma_start(cp[:, (q + 2) * V:(q + 3) * V],
                                      partial_b[q * 128:(q + 1) * 128, :])
                for q in range(2):
                    nc.sync.dma_start(rs_a[q * 128:(q + 1) * 128, :],
                                      cp[:, q * V:(q + 1) * V])
                    nc.sync.dma_start(rs_b[q * 128:(q + 1) * 128, :],
                                      cp[:, (q + 2) * V:(q + 3) * V])
            else:
                nc.gpsimd.collective_compute(
                    'ReduceScatter', mybir.AluOpType.add,
                    replica_groups=[list(range(NCORES))],
                    ins=[partial_b[:]], outs=[rs_b[:]])

            # ---- softmax over T per (batch, v) on the local 2-batch shard --
            with tc.tile_pool(name='ptail', bufs=1, space='PSUM') as ptailp:
                pts = ptailp.tile([64, BSH * T], f32, tag='tr')
                for q in range(BSH * T // 128):
                    sld = loads.tile([128, V], f32, tag='sld')
                    rs_src = rs_a if q < 2 else rs_b
                    nc.sync.dma_start(
                        sld[:], rs_src[(q % 2) * 128:(q % 2 + 1) * 128, :])
                    nc.tensor.transpose(pts[:, q * 128:(q + 1) * 128], sld[:],
                                        ident[:])
                sm = tailp.tile([64, BSH * T], f32)
                den = smallp.tile([64, BSH], f32, tag='den')
                for b in range(BSH):
                    nc.scalar.activation(
                        sm[:, b * T:(b + 1) * T], pts[:, b * T:(b + 1) * T],
                        mybir.ActivationFunctionType.Exp,
                        scale=1.0 / C, accum_out=den[:, b:b + 1])
                rden = smallp.tile([64, BSH], f32, tag='rden')
                nc.vector.reciprocal(rden[:], den[:])
                for b in range(BSH):
                    nc.vector.tensor_scalar(
                        out=sm[:, b * T:(b + 1) * T],
                        in0=sm[:, b * T:(b + 1) * T],
                        scalar1=rden[:, b:b + 1], scalar2=None,
                        op0=mybir.AluOpType.mult)
                pso = ptailp.tile([128, BSH * T // 128 * V], f32, tag='tro')
                so = tailp.tile([128, BSH * T // 128 * V], f32, tag='so')
                for q in range(BSH * T // 128):
                    nc.tensor.transpose(pso[:, q * V:(q + 1) * V],
                                        sm[:, q * 128:(q + 1) * 128],
                                        ident[0:64, 0:64])
                nc.scalar.copy(so[:], pso[:])
                for q in range(BSH * T // 128):
                    nc.sync.dma_start(out[q * 128:(q + 1) * 128, :],
                                      so[:, q * V:(q + 1) * V])

    nc.compile()
    return nc


def _get_program():
    if 'nc' not in _prog_cache:
        _prog_cache['nc'] = _build_program()
    return _prog_cache['nc']


def _split_f32r(a):
    """Split fp32 array into (hi, lo), both fp32r-representable (low 12
    mantissa bits zero), with hi + lo == a exactly."""
    bits = np.ascontiguousarray(a, dtype=np.float32).view(np.uint32)
    hi = (bits & np.uint32(0xFFFFF000)).view(np.float32)
    lo = a - hi
    return hi, lo


def kernel(batch, keys, values):
    from concourse import bass_utils

    nc = _get_program()
    ident = np.eye(128, dtype=np.float32)
    in_maps = []
    ones2 = np.ones((2, NT), dtype=np.float32)
    for c in range(NCORES):
        xT = np.ascontiguousarray(
            batch[:, c].reshape(NT, D).astype(np.float32).T)     # [64, NT]
        kT = np.ascontiguousarray(keys[c].astype(np.float32).T)  # [64, K]
        xr, xe = _split_f32r(xT)
        kr, ke = _split_f32r(kT)
        in_maps.append({
            'xsplit': np.ascontiguousarray(np.concatenate([xr, xe], axis=0)),
            'xt2src': np.ascontiguousarray(np.concatenate([xr, ones2], axis=0)),
            'krdup': np.ascontiguousarray(np.concatenate([kr, kr], axis=0)),
            'kesrc': np.ascontiguousarray(ke),
            'ksq': np.ascontiguousarray(kT * kT),
            'values': np.ascontiguousarray(values[c].astype(np.float32)),
            'ident': ident,
        })
    res = bass_utils.run_bass_kernel_spmd(nc, in_maps,
                                          core_ids=list(range(NCORES)))
    # core i holds batches {i, i + 8} (split reduce-scatter halves)
    out = np.empty((B, T, V), dtype=np.float32)
    for i in range(NCORES):
        shard = res.results[i]['out'].reshape(BSH, T, V)
        out[i] = shard[0]
        out[i + NCORES] = shard[1]
    return out


# revision 39
# speedup vs baseline: 2.5351x; 1.5189x over previous
"""DiscreteKeyValueBottleneck Trainium2 kernel.

Reference computation (per batch b, codebook c, token t):
  idx = argmin_k ||batch[b,c,t,:] - keys[c,k,:]||^2
  mapped[b,c,t,:] = values[c, idx, :]
  pooled = mean_c mapped               -> [B, T, V]
  out = softmax(pooled, axis=T)        -> [B, T, V]

Sharding: one codebook per NeuronCore (expert-style, C == 8 == n_cores).
Each core computes its codebook's mapped values for ALL batches, a
ReduceScatter(add) over the batch axis combines codebooks and leaves each
core with 2 batches, on which it runs the softmax locally.

argmin is computed as argmax_k (x.k - |k|^2/2).  The scores matmul runs
in fp32r (fp32 with 12 mantissa bits dropped; 1 PE cycle/row vs 4 for
fp32 when the moving dim is >= 256) using an exact 2-term split:
x = x_r + x_e and k = k_r + k_e with _r/_e both fp32r-representable, so
  x.k - |k|^2/2 = [x_r;x_e].[k_r;k_r]          (MM1, fp32r, 128-row)
                + [x_r';1;1;1].[k_e';k2a;k2b;k2c] (MM2, fp16, 67-row,
                  PSUM-accum; k_e' = k_e*2^12, x_r' = x_r*2^-12, |k|^2/2
                  as a 3-term fp16 split)
dropping only x_e.k_e ~ 2^-26/product (total score error ~1e-6; near-tie
key pairs with gaps ~1e-7 exist in this data where the fp32 reference's
own choice is rounding-arbitrary — output impact stays ~3e-4 of scale,
far under the 2e-2 gate, and is deterministic).  The r/e
splits, operand transposes, row duplication and -|k|^2/2 rows are
host-side preprocessing of the inputs (pure layout + a keys-only
constant); everything x-dependent heavy runs on device.

argmax runs as ONE custom DVE instruction per token tile (ARGMAX2_ANT,
registered into concourse.dve_ops at build time): in0/in1 = even/odd
elements of the score row, w = max(src0,src1), m = scan(MAX, w),
emit select(w >= m, 2*Idx + (src1>src0), -FLT_MAX), accum(MAX) ->
the argmax index in a single 2048-column pass (vs 8192 columns for
max8 + max_index).  8 ALU stages exactly.

Steady-state schedule (one 128-token tile, ~3.4us period, all of PE/
DVE/ACT ~80-85% busy): PE runs 16 fp32r matmuls (4 PSUM groups of 2
chunks so tile-granular WAR deps decouple); DVE copies group 0 to SBUF
then runs the fused argmax of the PREVIOUS tile (software-pipelined);
ACT copies groups 1-3; Pool gathers value rows by index (SWDGE); SP
streams operand loads + partial stores.  Two dummy matmuls at t=0 ramp
the PE p-state to full clock during the operand DMA stream-in.
"""

import numpy as np

B, C, T, D = 16, 8, 256, 64
K, V = 4096, 64
NCORES = 8
NT = B * T            # tokens per core (all batches, one codebook)
NTILES = NT // 128    # 32 token tiles
NCHUNK = K // 512     # 8 key chunks
BSH = B // NCORES     # batches per core after reduce-scatter

_prog_cache = {}


# ---------------------------------------------------------------------------
# custom DVE op: single-pass argmax over even/odd interleaved halves
# ---------------------------------------------------------------------------

def _ref_argmax2(in0, in1, c0, c1, c2):
    P = in0.shape[0]
    a = np.asarray(in0, np.float32).reshape(P, -1)
    b = np.asarray(in1, np.float32).reshape(P, -1)
    w = np.maximum(a, b)
    m = np.maximum.accumulate(w, axis=1)
    tag = (b > a).astype(np.float32)
    step = float(np.asarray(c0).flat[0]) if np.ndim(c0) else float(c0)
    idx2 = step * np.arange(w.shape[1], dtype=np.float32)[None, :]
    wi = idx2 + tag
    body = np.where(w >= m, wi, np.float32(np.finfo(np.float32).min))
    return body.astype(np.float32), body.max(axis=1, keepdims=True)


def _get_argmax_op():
    if 'op' in _prog_cache:
        return _prog_cache['op']
    from concourse.dve_spec import (
        Spec, Src0, Src1, C0, Zero, MaxNeg, AluOp, Bin, scan, select, maxx)
    from concourse.dve_ops import (
        DveOp, OPS, CUSTOM_DVE_SPECS, _SUB_OPCODE_FOR_NAME)
    from concourse.dve_uop import DveOpSpec
    from concourse.dve_spec import lower, _has_src1

    w = maxx(Src0, Src1)
    m = scan(AluOp.MAX, w)
    tag = Bin(AluOp.IS_LT, Src0, Src1)
    idx2 = scan(AluOp.ADD, C0, init=Bin(AluOp.SUBTRACT, Zero, C0))
    body = select(Bin(AluOp.IS_GE, w, m), idx2 + tag, MaxNeg)
    op = DveOp('ARGMAX2_ANT',
               Spec(body=body, accum=maxx, accum_init=MaxNeg,
                    reference=_ref_argmax2),
               subdim=False, uops_sha={})
    if op.name not in _SUB_OPCODE_FOR_NAME:
        OPS.append(op)
        CUSTOM_DVE_SPECS[op.name] = op.spec
        _SUB_OPCODE_FOR_NAME[op.name] = max(_SUB_OPCODE_FOR_NAME.values()) + 1
        for ver in ('v3', 'v4'):
            s = DveOpSpec(name=op.name, opcode=_SUB_OPCODE_FOR_NAME[op.name],
                          uops=lower(op.spec, ver=ver),
                          rd1_en=_has_src1(op.spec))
            op.uops_sha[ver] = s.sha(ver)
    _prog_cache['op'] = op
    return op


# ---------------------------------------------------------------------------
# bass program
# ---------------------------------------------------------------------------

def _build_program(single_core_sim=False):
    import concourse.bass as bass
    import concourse.tile as tile
    from concourse import bacc, mybir

    ARGMAX2 = _get_argmax_op()

    nc = bacc.Bacc('TRN2', target_bir_lowering=False, debug=False,
                   num_devices=1 if single_core_sim else NCORES)
    f32 = mybir.dt.float32
    f32r = mybir.dt.float32r
    u32 = mybir.dt.uint32

    # host-prepared, transposed + fp32r-split operands (see kernel()):
    #   xsplit  [128, NT]: rows 0-63 x_r^T, rows 64-127 x_e^T   (fp32r bits)
    #   xt2src  [67, NT]:  rows 0-63 fp16(x_r^T * 2^-12), rows 64-66 = 1.0
    #   krdup   [128, K]:  rows 0-63 k_r^T, rows 64-127 k_r^T
    #   kesrc   [67, K]:   rows 0-63 fp16(k_e^T * 2^12), rows 64-66 = 3-way
    #                      fp16 split of -|k|^2/2.  MM2 runs in fp16: its
    #                      terms are ~2^13 smaller than MM1's and the 2^+-12
    #                      pre-scaling (exact powers of two, cancelling in
    #                      the product) keeps k_e out of the fp16 subnormal
    #                      range -> total score error ~5e-7, well below the
    #                      min top-2 score gap.  Halves these loads' bytes
    #                      vs fp32r, pulling the last key quarter ~1.3us
    #                      earlier (the argmax pipeline start is gated on it)
    xsplit_d = nc.dram_tensor('xsplit', [128, NT], f32r, kind='ExternalInput').ap()
    xt2_d = nc.dram_tensor('xt2src', [67, NT], mybir.dt.float16,
                           kind='ExternalInput').ap()
    krdup_d = nc.dram_tensor('krdup', [128, K], f32r, kind='ExternalInput').ap()
    kesrc_d = nc.dram_tensor('kesrc', [67, K], mybir.dt.float16,
                             kind='ExternalInput').ap()
    values = nc.dram_tensor('values', [K, V], f32, kind='ExternalInput').ap()
    ident_in = nc.dram_tensor('ident', [128, 128], f32, kind='ExternalInput').ap()
    # out layout: [128, (BSH*T//128) * V] — row p, block q holds token
    # q*128 + p of the core's 2-batch shard; the host reorders (single DMA).
    out = nc.dram_tensor('out', [128, BSH * T // 128 * V], f32,
                         kind='ExternalOutput').ap()

    # two half-size bounce buffers so the first ReduceScatter (batches 0-7)
    # can launch while the second half of the main loop is still running;
    # each core ends up with batches {i, i+8} (host reorders).
    partial_a = nc.dram_tensor('partial_a', [NT // 2, V], f32).ap()
    partial_b = nc.dram_tensor('partial_b', [NT // 2, V], f32).ap()
    rs_a = nc.dram_tensor('rs_a', [T, V], f32).ap()
    rs_b = nc.dram_tensor('rs_b', [T, V], f32).ap()

    with tile.TileContext(nc) as tc:
        with (
            tc.tile_pool(name='const', bufs=1) as constp,
            tc.tile_pool(name='loads', bufs=3) as loads,
            tc.tile_pool(name='scores', bufs=3) as scoresp,
            tc.tile_pool(name='small', bufs=3) as smallp,
            tc.tile_pool(name='junk', bufs=1) as junkp,
            tc.tile_pool(name='prep', bufs=1) as prepp,
            tc.tile_pool(name='tail', bufs=1) as tailp,
        ):
            # ---- bulk operand loads, split + ordered so tile 0 can start
            # while the rest streams in (DMA engines serialize big loads) ----
            xsp = constp.tile([128, NT], f32r)
            xt2 = constp.tile([67, NT], mybir.dt.float16)
            krd = constp.tile([128, K], f32r)
            rhs2 = constp.tile([67, K], mybir.dt.float16)
            QK = K // 4
            QT = NT // 4
            ET = NT // 8
            nc.sync.dma_start(krd[:, 0:QK], krdup_d[:, 0:QK])
            nc.sync.dma_start(rhs2[:, 0:QK], kesrc_d[:, 0:QK])
            nc.sync.dma_start(xsp[:, 0:ET], xsplit_d[:, 0:ET])
            nc.sync.dma_start(xt2[:, 0:ET], xt2_d[:, 0:ET])
            for q in range(1, 4):
                nc.sync.dma_start(krd[:, q * QK:(q + 1) * QK],
                                  krdup_d[:, q * QK:(q + 1) * QK])
                nc.sync.dma_start(rhs2[:, q * QK:(q + 1) * QK],
                                  kesrc_d[:, q * QK:(q + 1) * QK])
            nc.sync.dma_start(xsp[:, ET:QT], xsplit_d[:, ET:QT])
            nc.sync.dma_start(xt2[:, ET:QT], xt2_d[:, ET:QT])
            for q in range(1, 4):
                nc.sync.dma_start(xsp[:, q * QT:(q + 1) * QT],
                                  xsplit_d[:, q * QT:(q + 1) * QT])
                nc.sync.dma_start(xt2[:, q * QT:(q + 1) * QT],
                                  xt2_d[:, q * QT:(q + 1) * QT])
            ident = constp.tile([128, 128], f32)
            nc.sync.dma_start(ident[:], ident_in[:])
            idxall = constp.tile([128, NTILES], f32)
            sldA = constp.tile([128, 2 * V], f32)
            ones64 = constp.tile([64, 1], f32)
            nc.vector.memset(ones64[:], 1.0)
            warm = constp.tile([64, 512], f32)
            nc.vector.memset(warm[:, 0:1], 1.0)

            # ---- main loop ----
            # Software-pipelined: tile t's matmuls+copies are emitted
            # together with tile t-1's argmax/gather so the DVE's early
            # chunk-0 copy (bank-disjoint from ACT's copies) never couples
            # into the argmax chain.
            scores_t = [None] * NTILES

            def emit_post(tp):
                junk = junkp.tile([128, K // 2], f32)
                nc.vector._custom_dve(
                    ARGMAX2, out=junk[:],
                    in0=scores_t[tp][:, 0:K:2], in1=scores_t[tp][:, 1:K:2],
                    s0=2.0, accum_out=idxall[:, tp:tp + 1])
                idxu = smallp.tile([128, 1], u32, tag='idxu')
                nc.vector.tensor_copy(out=idxu[:], in_=idxall[:, tp:tp + 1])
                mapped = smallp.tile([128, V], f32, tag='mapped')
                nc.gpsimd.indirect_dma_start(
                    out=mapped[:], out_offset=None, in_=values[:],
                    in_offset=bass.IndirectOffsetOnAxis(ap=idxu[:], axis=0))
                if tp < NTILES // 2:
                    nc.sync.dma_start(
                        partial_a[tp * 128:(tp + 1) * 128, :], mapped[:])
                else:
                    tt = tp - NTILES // 2
                    nc.sync.dma_start(
                        partial_b[tt * 128:(tt + 1) * 128, :], mapped[:])
                if tp == 20:
                    # rs_a (first-half ReduceScatter output) is final since
                    # tile 15; pull its softmax input loads out of the
                    # congested closing HWDGE window
                    nc.sync.dma_start(sldA[:, 0:V], rs_a[0:128, :])
                    nc.sync.dma_start(sldA[:, V:2 * V], rs_a[128:256, :])
                if tp == NTILES // 2 - 1:
                    if single_core_sim:
                        cp = tailp.tile([128, 2 * V], f32, tag='rscopy_a')
                        for q in range(2):
                            nc.sync.dma_start(cp[:, q * V:(q + 1) * V],
                                              partial_a[q * 128:(q + 1) * 128, :])
                        for q in range(2):
                            nc.sync.dma_start(rs_a[q * 128:(q + 1) * 128, :],
                                              cp[:, q * V:(q + 1) * V])
                    else:
                        nc.gpsimd.collective_compute(
                            'ReduceScatter', mybir.AluOpType.add,
                            replica_groups=[list(range(NCORES))],
                            ins=[partial_a[:]], outs=[rs_a[:]])

            with tc.tile_pool(name='pmm', bufs=1, space='PSUM') as pmm:
                # dummy matmuls ramp the PE clock to full p-state while the
                # operand DMAs stream in (outputs never read)
                pmw = pmm.tile([128, 1024], f32, tag='mm3')
                for w in range(2):
                    nc.tensor.matmul(pmw[0:1, 0:512], ones64[:], warm[:],
                                     start=True, stop=True)
                nc.tensor.matmul(pmw[0:1, 0:256], ones64[:], warm[:, 0:256],
                                 start=True, stop=True)
                for t in range(NTILES):
                    scores = scoresp.tile([128, K], f32, tag='scores')
                    scores_t[t] = scores
                    lhsT1 = xsp[:, t * 128:(t + 1) * 128]
                    lhsT2 = xt2[:, t * 128:(t + 1) * 128]
                    # PSUM groups sized so WAR deps decouple (dep tracking
                    # is tile-granular): [c0] -> DVE, [c1-3]/[c4-7] -> ACT.
                    def mmpair(pm, col, j):
                        nc.tensor.matmul(pm[:, col:col + 512], lhsT1,
                                         krd[:, j * 512:(j + 1) * 512],
                                         start=True, stop=False)
                        nc.tensor.matmul(pm[:, col:col + 512], lhsT2,
                                         rhs2[:, j * 512:(j + 1) * 512],
                                         start=False, stop=True)
                    for g in range(4):
                        pm = pmm.tile([128, 1024], f32, tag=f'mm{g}')
                        mmpair(pm, 0, 2 * g)
                        mmpair(pm, 512, 2 * g + 1)
                        if g == 0:
                            nc.vector.tensor_copy(out=scores[:, 0:1024],
                                                  in_=pm[:])
                        else:
                            nc.scalar.copy(
                                scores[:, g * 1024:(g + 1) * 1024], pm[:])
                    if t > 0:
                        emit_post(t - 1)
                emit_post(NTILES - 1)

            # ---- combine codebooks: second-half ReduceScatter ----
            if single_core_sim:
                # TimelineSim can't simulate collectives; stand in same-size
                # local copies so the tail still gets modeled.
                cp = tailp.tile([128, 2 * V], f32, tag='rscopy_b')
                for q in range(2):
                    nc.sync.dma_start(cp[:, q * V:(q + 1) * V],
                                      partial_b[q * 128:(q + 1) * 128, :])
                for q in range(2):
                    nc.sync.dma_start(rs_b[q * 128:(q + 1) * 128, :],
                                      cp[:, q * V:(q + 1) * V])
            else:
                nc.gpsimd.collective_compute(
                    'ReduceScatter', mybir.AluOpType.add,
                    replica_groups=[list(range(NCORES))],
                    ins=[partial_b[:]], outs=[rs_b[:]])

            # ---- softmax over T per (batch, v) on the local 2-batch shard --
            with tc.tile_pool(name='ptail', bufs=1, space='PSUM') as ptailp:
                pts = ptailp.tile([64, BSH * T], f32, tag='tr')
                sld = loads.tile([128, 2 * V], f32, tag='sld')
                for q in range(2):
                    nc.sync.dma_start(
                        sld[:, q * V:(q + 1) * V],
                        rs_b[q * 128:(q + 1) * 128, :])
                for q in range(BSH * T // 128):
                    srct = sldA if q < 2 else sld
                    nc.tensor.transpose(pts[:, q * 128:(q + 1) * 128],
                                        srct[:, (q % 2) * V:(q % 2 + 1) * V],
                                        ident[:])
                sm = tailp.tile([64, BSH * T], f32)
                den = smallp.tile([64, BSH], f32, tag='den')
                for b in range(BSH):
                    nc.scalar.activation(
                        sm[:, b * T:(b + 1) * T], pts[:, b * T:(b + 1) * T],
                        mybir.ActivationFunctionType.Exp,
                        scale=1.0 / C, accum_out=den[:, b:b + 1])
                rden = smallp.tile([64, BSH], f32, tag='rden')
                nc.vector.reciprocal(rden[:], den[:])
                for b in range(BSH):
                    nc.vector.tensor_scalar(
                        out=sm[:, b * T:(b + 1) * T],
                        in0=sm[:, b * T:(b + 1) * T],
                        scalar1=rden[:, b:b + 1], scalar2=None,
                        op0=mybir.AluOpType.mult)
                pso = ptailp.tile([128, BSH * T // 128 * V], f32, tag='tro')
                so = tailp.tile([128, BSH * T // 128 * V], f32, tag='so')
                for q in range(BSH * T // 128):
                    nc.tensor.transpose(pso[:, q * V:(q + 1) * V],
                                        sm[:, q * 128:(q + 1) * 128],
                                        ident[0:64, 0:64])
                nc.vector.tensor_copy(out=so[:], in_=pso[:])
                nc.sync.dma_start(out[:], so[:])

    nc.compile()
    return nc


def _get_program():
    if 'nc' not in _prog_cache:
        _prog_cache['nc'] = _build_program()
    return _prog_cache['nc']


def _split_f32r(a):
    """Split fp32 array into (hi, lo), both fp32r-representable (low 12
    mantissa bits zero), with hi + lo == a exactly."""
    bits = np.ascontiguousarray(a, dtype=np.float32).view(np.uint32)
    hi = (bits & np.uint32(0xFFFFF000)).view(np.float32)
    lo = a - hi
    return hi, lo


def _split_f16_3(a):
    """3-term fp16 split: a ~= b0 + b1 + b2 with each term fp16 (33
    mantissa bits total)."""
    b0 = a.astype(np.float16).astype(np.float32)
    r1 = a - b0
    b1 = r1.astype(np.float16).astype(np.float32)
    b2 = (r1 - b1).astype(np.float16).astype(np.float32)
    return b0, b1, b2


def kernel(batch, keys, values):
    from concourse import bass_utils

    batch = np.asarray(batch)
    keys = np.asarray(keys)
    values = np.asarray(values)
    nc = _get_program()
    ident = np.eye(128, dtype=np.float32)
    in_maps = []
    ones2 = np.ones((2, NT), dtype=np.float32)
    for c in range(NCORES):
        xT = np.ascontiguousarray(
            batch[:, c].reshape(NT, D).astype(np.float32).T)     # [64, NT]
        kT = np.ascontiguousarray(keys[c].astype(np.float32).T)  # [64, K]
        xr, xe = _split_f32r(xT)
        kr, ke = _split_f32r(kT)
        k2 = (-0.5 * (kT.astype(np.float64) ** 2).sum(axis=0)).astype(
            np.float32)[None, :]
        k2a, k2b, k2c = _split_f16_3(k2)
        xt2src = np.concatenate(
            [xr * np.float32(2.0 ** -12), ones2,
             np.ones((1, NT), np.float32)], axis=0).astype(np.float16)
        kesrc = np.concatenate(
            [ke * np.float32(2.0 ** 12), k2a, k2b, k2c],
            axis=0).astype(np.float16)
        in_maps.append({
            'xsplit': np.ascontiguousarray(np.concatenate([xr, xe], axis=0)),
            'xt2src': np.ascontiguousarray(xt2src),
            'krdup': np.ascontiguousarray(np.concatenate([kr, kr], axis=0)),
            'kesrc': np.ascontiguousarray(kesrc),
            'values': np.ascontiguousarray(values[c].astype(np.float32)),
            'ident': ident,
        })
    res = bass_utils.run_bass_kernel_spmd(nc, in_maps,
                                          core_ids=list(range(NCORES)))
    # core i holds batches {i, i + 8} (split reduce-scatter halves)
    out = np.empty((B, T, V), dtype=np.float32)
    for i in range(NCORES):
        arr = res.results[i]['out'].reshape(128, BSH * T // 128, V)
        shard = np.transpose(arr, (1, 0, 2)).reshape(BSH, T, V)
        out[i] = shard[0]
        out[i + NCORES] = shard[1]
    return out
